# revision 22
# baseline (speedup 1.0000x reference)
"""Trainium2 Bass kernel for nn_DistanceLayer (gaussian-prior distance attention).

Math: out[b,i] = sum_j softmax_j(q_i.k_j * MD^-0.5 * prior(j-i))[j] * (j-i)

The gaussian prior (std=1) underflows so fast in f32 that outside a small
band of offsets the f32 score is exactly 0, so exp(score) is exactly 1.0.
Each softmax row is a narrow band of interesting values plus a uniform far
field with closed-form sums.  We compute a narrow window of scores around
the diagonal on the PE and fold the far field in with exact constants:

    T0_i = (N - win) + sum_win e           (denominator)
    out_i = (A_i + sum_win e*j0 + B_i * sum_win e) / T0_i

with A_i = sum_all j - sum_win_i j - i*(N-win)  (exact ints in f32),
B_i = ws_i - i, j0 the window-local column index.

Structure:
- rows processed as 64-row halves packed two-per-partition-dim (windows
  stay narrow: win = 64 + band + pad); the h0/h64 half matmuls run
  concurrently on the PE via column groups.
- fp8 (e4m3) x and weights; each projection chunk is one DoubleRow matmul
  per q/k contracting all 256 input dims at 2 elem/cell; q and k land in
  one [P, 1024] PSUM tile, k evicted on ACT, q on DVE.
- postproc per QUAD of tiles [P, 4*win] to amortize fixed op costs:
  DVE multiplies scores by the premultiplied prior pattern into PSUM,
  ACT exp's the quad into a packed bf16 e|ej tile, DVE multiplies e by
  j0 into the ej half at 2x bf16 rate, and one DVE reduce over
  [P, (8, win)] yields all eight sums per quad.
- combine runs on GpSimd except the reciprocal; y is written in 2 DMAs.
- inputs split over the sync + scalar DMA queues ordered by need time.

Sharding: pure data-parallel over batch B=8 across the 8 cores.
"""

import sys

sys.path.insert(0, "/opt/trn_rl_repo")

import ml_dtypes
import numpy as np

import concourse.bacc as bacc
import concourse.tile as tile
from concourse import mybir
from concourse.bass_utils import run_bass_kernel_spmd

B, N, D, MD = 8, 2048, 256, 128
NCORES = 8
P = 128
HR = P // 2  # 64-row half-tiles
NT = N // P  # 16 row tiles
TPQ = 4  # tiles per postprocessing quad
NQUAD = NT // TPQ  # 4
DCH = D // P  # 2 contraction chunks (fused by DoubleRow)
PROJ_CHUNK = 512
NPC = N // PROJ_CHUNK  # 4 projection column chunks
PI = 3.1415926  # matches reference
F32 = mybir.dt.float32
BF16 = mybir.dt.bfloat16
F8 = mybir.dt.float8e4
AL = mybir.AluOpType
AF = mybir.ActivationFunctionType
N_WARM = 22  # PE clock-ramp junk matmuls
PAD = 16  # zero border around kT for unclamped windows

_cache = {}
# exposed for test harness profiling: (nc, in_maps)
last_run = None


def _plan_band(prior_mean, prior_std):
    """f32 prior over every offset, exactly as the reference computes it,
    and the band of offsets whose scores can round exp() away from 1.0."""
    d = np.arange(-(N - 1), N, dtype=np.float32)
    ps = np.float32(prior_std)
    pm = np.float32(prior_mean)
    prior = (
        np.float32(1.0)
        / ps
        / np.sqrt(np.float32(2.0) * np.float32(PI))
        * np.exp(np.float32(-0.5) * (d - pm) ** 2 / ps**2)
    ).astype(np.float32)
    sig = np.abs(prior) * 1024.0 >= 2.0**-27
    if not sig.any():
        dlo, dhi = 0, 0
    else:
        dlo = int(d[sig].min())
        dhi = int(d[sig].max())
    return prior, dlo, dhi


def _window_geometry(dlo, dhi):
    """Per-64-row-half window starts ws2[32] plus deduplicated per-quad
    prior patterns.  Pattern key for quad q (tiles 4q..4q+3) is the tuple
    of its eight half-window offsets relative to the quad's base row."""
    span = dhi - dlo
    win = HR + span + 1
    win = max(80, ((win + 15) // 16) * 16)
    assert win <= 128, f"prior band too wide for banded kernel: {dlo}..{dhi}"
    extra = win - (HR + span)
    # windows are deliberately UNCLAMPED: out-of-range columns read the
    # zero padding around kT, giving score 0 and exp(0)=1, which the
    # far-field constants absorb exactly.  This makes every quad share
    # one prior pattern.
    ws2 = []
    for h in range(2 * NT):
        ws = h * HR + dlo - extra // 2
        lo_need = max(0, h * HR + dlo)
        hi_need = min(N - 1, h * HR + HR - 1 + dhi)
        assert ws <= lo_need and hi_need < ws + win, (h, ws, lo_need, hi_need)
        ws2.append(ws)
    quad_keys = []
    for q in range(NQUAD):
        base = TPQ * P * q
        quad_keys.append(tuple(ws2[2 * TPQ * q + i] - base for i in range(2 * TPQ)))
    key_vals = sorted(set(quad_keys))
    key_idx = [key_vals.index(k) for k in quad_keys]
    return win, ws2, key_vals, key_idx


def _build(win, ws2, key_idx, n_pat, has_bias):
    nc = bacc.Bacc()

    W4 = TPQ * win  # postproc pass width
    CW = 2 + 2 * NT
    O_BQ, O_BK = 0, 1
    O_A = 2
    O_B = O_A + NT
    O_J0 = win  # j0 offset inside cst16

    w2_d = nc.dram_tensor("w2", [P, 2 * DCH * MD], F8, kind="ExternalInput")
    xt_d = nc.dram_tensor("xt", [NPC, P, DCH * PROJ_CHUNK], F8, kind="ExternalInput")
    cs_d = nc.dram_tensor("cst", [P, CW], F32, kind="ExternalInput")
    c16_d = nc.dram_tensor("cst16", [P, win + TPQ * win], BF16, kind="ExternalInput")
    y_d = nc.dram_tensor("y", [P, NT], F32, kind="ExternalOutput")

    with tile.TileContext(nc) as tc:
        with (
            tc.tile_pool(name="const", bufs=1) as const,
            tc.tile_pool(name="psum_proj", bufs=2, space="PSUM") as psum_proj,
            tc.tile_pool(name="psum_band", bufs=3, space="PSUM") as psum_band,
            tc.tile_pool(name="band_sp", bufs=3) as sp_pool,
            tc.tile_pool(name="band_e", bufs=3) as e_pool,
            tc.tile_pool(name="comb", bufs=1) as comb,
        ):
            # ---- engine warmups (run while DMAs are in flight) ----
            wtile = const.tile([P, 64], BF16, tag="warm_w")
            nc.vector.memset(wtile, 0.0)
            wact_in = const.tile([P, 1], F32, tag="warm_a")
            nc.vector.memset(wact_in, 0.0)
            for _ in range(N_WARM):
                wps = psum_band.tile([P, W4], F32, tag="band")
                nc.tensor.matmul(
                    wps[:64, :64], lhsT=wtile[:, :64], rhs=wtile, start=True, stop=True
                )
            wact_out = const.tile([P, 1], F32, tag="warm_ao")
            nc.scalar.activation(out=wact_out, in_=wact_in, func=AF.Exp)

            # ---- input DMAs, ordered by need time ----
            # sync queue: xt0, xt2, cst16, cst (+ the y output later)
            # scalar queue: w2, xt1, xt3
            xts = []
            for i in range(NPC):
                t = const.tile([P, DCH * PROJ_CHUNK], F8, tag=f"xt{i}")
                xts.append(t)
            nc.sync.dma_start(out=xts[0], in_=xt_d[0])
            w2_s = const.tile([P, 2 * DCH * MD], F8, tag="w2")
            nc.scalar.dma_start(out=w2_s, in_=w2_d[:, :])
            nc.sync.dma_start(out=xts[1], in_=xt_d[1])
            nc.scalar.dma_start(out=xts[2], in_=xt_d[2])
            c16_s = const.tile([P, win + W4], BF16, tag="cst16")
            nc.sync.dma_start(out=c16_s, in_=c16_d[:, :])
            nc.scalar.dma_start(out=xts[3], in_=xt_d[3])
            cs_s = const.tile([P, CW], F32, tag="cst")
            nc.sync.dma_start(out=cs_s, in_=cs_d[:, :])

            # q is cols [0, N), k is cols [N+PAD, 2N+PAD) with zeroed
            # PAD-wide borders so unclamped windows read exp-neutral zeros
            qkT = const.tile([P, 2 * N + 2 * PAD], BF16, tag="qkT")
            nc.gpsimd.memset(qkT[:, N : N + PAD], 0.0)
            nc.gpsimd.memset(qkT[:, 2 * N + PAD :], 0.0)
            # per-quad sums, 8 cols per quad: e(4q..4q+3) | ec(4q..4q+3)
            sums8 = const.tile([P, 2 * TPQ * NQUAD], F32, tag="sums8")
            outv = const.tile([P, NT], F32, tag="outv")

            # ---- projections ----
            def emit_proj(n4):
                ps = psum_proj.tile([P, 2 * PROJ_CHUNK], F32, tag="proj")
                rhs = xts[n4][:, :].rearrange("p (c f) -> p c f", c=DCH)
                for pj in range(2):  # 0=q, 1=k
                    lhsT = w2_s[:, pj * DCH * MD : (pj + 1) * DCH * MD].rearrange(
                        "p (c m) -> p c m", c=DCH
                    )
                    nc.tensor.matmul(
                        ps[:, pj * PROJ_CHUNK : (pj + 1) * PROJ_CHUNK],
                        lhsT=lhsT,
                        rhs=rhs,
                        start=True,
                        stop=True,
                        perf_mode=mybir.MatmulPerfMode.DoubleRow,
                    )
                return ps

            def emit_evict(n4, ps, pj, eng):
                lo = n4 * PROJ_CHUNK
                src = ps[:, pj * PROJ_CHUNK : (pj + 1) * PROJ_CHUNK]
                ko = pj * (N + PAD)
                dst = qkT[:, ko + lo : ko + lo + PROJ_CHUNK]
                if has_bias:
                    nc.scalar.activation(
                        out=dst, in_=src, func=AF.Identity,
                        bias=cs_s[:, O_BQ + pj : O_BQ + pj + 1], scale=1.0,
                    )
                elif eng == "act":
                    nc.scalar.activation(out=dst, in_=src, func=AF.Identity)
                else:
                    nc.vector.tensor_copy(dst, src)

            # ---- band quad: tiles 4q..4q+3 share one [P, 4*win] pass ----
            quad_ps = {}

            def emit_quad_mm(q):
                ps = psum_band.tile([P, W4], F32, tag="band")
                for tb in range(TPQ):
                    t = TPQ * q + tb
                    for hb in range(2):  # 64-row half on partitions
                        ws = ws2[2 * t + hb]
                        nc.tensor.matmul(
                            ps[hb * HR : (hb + 1) * HR, tb * win : (tb + 1) * win],
                            lhsT=qkT[:, t * P + hb * HR : t * P + (hb + 1) * HR],
                            rhs=qkT[:, N + PAD + ws : N + PAD + ws + win],
                            start=True,
                            stop=True,
                        )
                quad_ps[q] = ps

            quad_eej = {}

            def emit_quad_red(q):
                eej = quad_eej.pop(q)
                nc.vector.tensor_reduce(
                    out=sums8[:, 2 * TPQ * q : 2 * TPQ * (q + 1)],
                    in_=eej[:].rearrange("p (f w) -> p f w", w=win),
                    axis=mybir.AxisListType.X,
                    op=AL.add,
                )

            def emit_quad_post(q):
                ps = quad_ps.pop(q)
                sp = sp_pool.tile([P, W4], F32, tag="sp")
                pat_b = (
                    c16_s[:, :win].unsqueeze(1).broadcast_to([P, TPQ, win])
                )
                nc.vector.tensor_mul(
                    sp[:].rearrange("p (f w) -> p f w", w=win),
                    ps[:].rearrange("p (f w) -> p f w", w=win),
                    pat_b,
                )
                # packed e | ej tile: exp writes [:, :W4], ej in [:, W4:]
                eej = e_pool.tile([P, 2 * W4], BF16, tag="eej")
                nc.scalar.activation(out=eej[:, :W4], in_=sp, func=AF.Exp)
                nc.vector.tensor_mul(
                    eej[:, W4:], eej[:, :W4], c16_s[:, O_J0 : O_J0 + W4]
                )
                quad_eej[q] = eej

            # ---- combine: out = (A + sum_ec + B*sum_e)/(N-win+sum_e) ----
            # sums8 views: per quad 8 cols [e x4, ec x4]
            def sview(sl, off):
                q0, q1 = sl.start // TPQ, sl.stop // TPQ
                return sums8[:, 2 * TPQ * q0 : 2 * TPQ * q1].rearrange(
                    "p (q eight) -> p q eight", eight=2 * TPQ
                )[:, :, off : off + TPQ]

            def cview(o, sl):
                return cs_s[:, o + sl.start : o + sl.stop].rearrange(
                    "p (q four) -> p q four", four=TPQ
                )

            def comb_pre(sl):
                w = sl.stop - sl.start
                se = sview(sl, 0)
                t0 = comb.tile([P, w], F32, tag=f"t0{sl.start}")
                nc.gpsimd.tensor_scalar_add(
                    t0[:].rearrange("p (q four) -> p q four", four=TPQ),
                    se,
                    float(N - win),
                )
                rec = comb.tile([P, w], F32, tag=f"rec{sl.start}")
                nc.vector.reciprocal(rec, t0)
                tmp = comb.tile([P, w], F32, tag=f"tmp{sl.start}")
                nc.gpsimd.tensor_mul(
                    tmp[:].rearrange("p (q four) -> p q four", four=TPQ),
                    cview(O_B, sl),
                    se,
                )
                return rec, tmp

            def comb_post(sl, rec, tmp):
                w = sl.stop - sl.start
                sec = sview(sl, TPQ)
                num = comb.tile([P, w], F32, tag=f"num{sl.start}")
                nc.gpsimd.tensor_add(
                    num[:].rearrange("p (q four) -> p q four", four=TPQ),
                    cview(O_A, sl),
                    sec,
                )
                num2 = comb.tile([P, w], F32, tag=f"num2{sl.start}")
                nc.gpsimd.tensor_add(num2, num, tmp)
                nc.gpsimd.tensor_mul(outv[:, sl], num2, rec)
                nc.sync.dma_start(out=y_d[:, sl], in_=outv[:, sl])

            # ---- schedule ----
            slA, slB = slice(0, 8), slice(8, NT)

            ps0 = emit_proj(0)
            emit_evict(0, ps0, 1, "act")
            emit_evict(0, ps0, 0, "dve")
            ps1 = emit_proj(1)
            emit_evict(1, ps1, 1, "act")
            emit_evict(1, ps1, 0, "dve")
            ps2 = emit_proj(2)
            emit_evict(2, ps2, 1, "act")
            emit_quad_mm(0)
            ps3 = emit_proj(3)
            emit_evict(2, ps2, 0, "dve")
            emit_evict(3, ps3, 1, "act")
            emit_evict(3, ps3, 0, "dve")
            emit_quad_post(0)
            emit_quad_mm(1)
            emit_quad_red(0)
            emit_quad_post(1)
            emit_quad_mm(2)
            emit_quad_post(2)
            emit_quad_red(1)
            emit_quad_mm(3)
            emit_quad_post(3)
            emit_quad_red(2)
            recA, tmpA = comb_pre(slA)
            comb_post(slA, recA, tmpA)
            emit_quad_red(3)
            recB, tmpB = comb_pre(slB)
            comb_post(slB, recB, tmpB)

    nc.finalize()
    return nc


def kernel(x, Wq, bq, Wk, bk, prior_mean, prior_std):
    global last_run
    x = np.asarray(x, dtype=np.float32)
    Wq = np.asarray(Wq, dtype=np.float32)
    Wk = np.asarray(Wk, dtype=np.float32)
    bq = np.asarray(bq, dtype=np.float32)
    bk = np.asarray(bk, dtype=np.float32)
    has_bias = bool(np.any(bq) or np.any(bk))

    prior, dlo, dhi = _plan_band(
        float(np.asarray(prior_mean)[0]), float(np.asarray(prior_std)[0])
    )
    win, ws2, key_vals, key_idx = _window_geometry(dlo, dhi)
    n_pat = len(key_vals)

    key = (win, tuple(ws2), tuple(key_idx), has_bias)
    if key not in _cache:
        _cache[key] = _build(win, ws2, key_idx, n_pat, has_bias)
    nc = _cache[key]

    bf = ml_dtypes.bfloat16
    f8 = ml_dtypes.float8_e4m3fn
    scale = np.float32(MD**-0.5)
    W4 = TPQ * win

    # prior*scale pattern: with unclamped windows every tile block is the
    # same [P, win] pattern: dm = c + dlo - extra//2 + HR*(p>=HR) - p
    p_idx = np.arange(P)[:, None]
    c_idx = np.arange(win)[None, :]
    rel0 = ws2[0]  # = dlo - extra//2
    relcol = np.where(np.arange(P) < HR, rel0, rel0 + HR)[:, None]
    dm = c_idx + relcol - p_idx
    pmat = np.where(
        (dm >= dlo) & (dm <= dhi), prior[dm + N - 1] * scale, np.float32(0.0)
    ).astype(np.float32)

    sumj_all = float(N * (N - 1) // 2)
    half_sel = np.arange(P) >= HR
    ii = (np.arange(P)[:, None] + P * np.arange(NT)[None, :]).astype(np.float32)
    wsv = np.zeros((P, NT), np.float32)
    for t in range(NT):
        wsv[:, t] = np.where(half_sel, float(ws2[2 * t + 1]), float(ws2[2 * t]))
    c1 = sumj_all - (win * wsv + win * (win - 1) // 2)
    A = c1 - ii * float(N - win)
    Bv = wsv - ii

    cst = np.ascontiguousarray(
        np.concatenate([bq.reshape(P, 1), bk.reshape(P, 1), A, Bv], axis=1).astype(
            np.float32
        )
    )
    j0quad = np.broadcast_to(
        np.tile(np.arange(win, dtype=np.float32), TPQ)[None, :], (P, TPQ * win)
    )
    cst16 = np.ascontiguousarray(np.concatenate([pmat, j0quad], axis=1).astype(bf))

    # weights: wq chunks then wk chunks, [P, 4*MD] fp8
    wq_h = Wq.reshape(DCH, P, MD).transpose(1, 0, 2).reshape(P, DCH * MD)
    wk_h = Wk.reshape(DCH, P, MD).transpose(1, 0, 2).reshape(P, DCH * MD)
    w2_h = np.clip(np.concatenate([wq_h, wk_h], axis=1), -240, 240)
    w2_h = np.ascontiguousarray(w2_h).astype(f8)

    in_maps = []
    for core in range(NCORES):
        xb = x[core]  # [N, D]
        # xt[n4, p, c*512 + j] = x[n4*512 + j, c*128 + p]
        xt_h = np.ascontiguousarray(
            np.clip(xb.T, -240, 240)
            .reshape(DCH, P, NPC, PROJ_CHUNK)
            .transpose(2, 1, 0, 3)
            .reshape(NPC, P, DCH * PROJ_CHUNK)
        ).astype(f8)
        in_maps.append({"xt": xt_h, "w2": w2_h, "cst": cst, "cst16": cst16})

    res = run_bass_kernel_spmd(nc, in_maps, list(range(NCORES)))
    last_run = (nc, in_maps)
    # y[p, t] = out[128t + p]  ->  out = y.T.flatten()
    out = np.stack(
        [res.results[c]["y"].T.reshape(-1) for c in range(NCORES)], axis=0
    )
    return out.astype(np.float32)


# revision 23
# speedup vs baseline: 1.0190x; 1.0190x over previous
"""Trainium2 Bass kernel for nn_DistanceLayer (gaussian-prior distance attention).

Math: out[b,i] = sum_j softmax_j(q_i.k_j * MD^-0.5 * prior(j-i))[j] * (j-i)

The gaussian prior (std=1) underflows so fast in f32 that outside a small
band of offsets the f32 score is exactly 0, so exp(score) is exactly 1.0.
Each softmax row is a narrow band of interesting values plus a uniform far
field with closed-form sums.  We compute a narrow window of scores around
the diagonal on the PE and fold the far field in with exact constants:

    T0_i = (N - win) + sum_win e           (denominator)
    out_i = (A_i + sum_win e*j0 + B_i * sum_win e) / T0_i

with A_i = sum_all j - sum_win_i j - i*(N-win)  (exact ints in f32),
B_i = ws_i - i, j0 the window-local column index.

Structure:
- rows processed as 64-row halves packed two-per-partition-dim (windows
  stay narrow: win = 64 + band + pad); the h0/h64 half matmuls run
  concurrently on the PE via column groups.
- fp8 (e4m3) x and weights; each projection chunk is one DoubleRow matmul
  per q/k contracting all 256 input dims at 2 elem/cell; q and k land in
  one [P, 1024] PSUM tile, k evicted on ACT, q on DVE.
- postproc per QUAD of tiles [P, 4*win] to amortize fixed op costs:
  DVE multiplies scores by the premultiplied prior pattern into PSUM,
  ACT exp's the quad into a packed bf16 e|ej tile, DVE multiplies e by
  j0 into the ej half at 2x bf16 rate, and one DVE reduce over
  [P, (8, win)] yields all eight sums per quad.
- combine runs on GpSimd except the reciprocal; y is written in 2 DMAs.
- inputs split over the sync + scalar DMA queues ordered by need time.

Sharding: pure data-parallel over batch B=8 across the 8 cores.
"""

import sys

sys.path.insert(0, "/opt/trn_rl_repo")

import ml_dtypes
import numpy as np

import concourse.bacc as bacc
import concourse.tile as tile
from concourse import mybir
from concourse.bass_utils import run_bass_kernel_spmd

B, N, D, MD = 8, 2048, 256, 128
NCORES = 8
P = 128
HR = P // 2  # 64-row half-tiles
NT = N // P  # 16 row tiles
TPQ = 4  # tiles per postprocessing quad
NQUAD = NT // TPQ  # 4
DCH = D // P  # 2 contraction chunks (fused by DoubleRow)
PROJ_CHUNK = 512
NPC = N // PROJ_CHUNK  # 4 projection column chunks
PI = 3.1415926  # matches reference
F32 = mybir.dt.float32
BF16 = mybir.dt.bfloat16
F8 = mybir.dt.float8e4
AL = mybir.AluOpType
AF = mybir.ActivationFunctionType
N_WARM = 22  # PE clock-ramp junk matmuls
PAD = 16  # zero border around kT for unclamped windows

_cache = {}
# exposed for test harness profiling: (nc, in_maps)
last_run = None


def _plan_band(prior_mean, prior_std):
    """f32 prior over every offset, exactly as the reference computes it,
    and the band of offsets whose scores can round exp() away from 1.0."""
    d = np.arange(-(N - 1), N, dtype=np.float32)
    ps = np.float32(prior_std)
    pm = np.float32(prior_mean)
    prior = (
        np.float32(1.0)
        / ps
        / np.sqrt(np.float32(2.0) * np.float32(PI))
        * np.exp(np.float32(-0.5) * (d - pm) ** 2 / ps**2)
    ).astype(np.float32)
    sig = np.abs(prior) * 1024.0 >= 2.0**-27
    if not sig.any():
        dlo, dhi = 0, 0
    else:
        dlo = int(d[sig].min())
        dhi = int(d[sig].max())
    return prior, dlo, dhi


def _window_geometry(dlo, dhi):
    """Per-64-row-half window starts ws2[32] plus deduplicated per-quad
    prior patterns.  Pattern key for quad q (tiles 4q..4q+3) is the tuple
    of its eight half-window offsets relative to the quad's base row."""
    span = dhi - dlo
    win = HR + span + 1
    win = max(80, ((win + 15) // 16) * 16)
    assert win <= 128, f"prior band too wide for banded kernel: {dlo}..{dhi}"
    extra = win - (HR + span)
    # windows are deliberately UNCLAMPED: out-of-range columns read the
    # zero padding around kT, giving score 0 and exp(0)=1, which the
    # far-field constants absorb exactly.  This makes every quad share
    # one prior pattern.
    ws2 = []
    for h in range(2 * NT):
        ws = h * HR + dlo - extra // 2
        lo_need = max(0, h * HR + dlo)
        hi_need = min(N - 1, h * HR + HR - 1 + dhi)
        assert ws <= lo_need and hi_need < ws + win, (h, ws, lo_need, hi_need)
        ws2.append(ws)
    quad_keys = []
    for q in range(NQUAD):
        base = TPQ * P * q
        quad_keys.append(tuple(ws2[2 * TPQ * q + i] - base for i in range(2 * TPQ)))
    key_vals = sorted(set(quad_keys))
    key_idx = [key_vals.index(k) for k in quad_keys]
    return win, ws2, key_vals, key_idx


def _build(win, ws2, key_idx, n_pat, has_bias):
    nc = bacc.Bacc()

    W4 = TPQ * win  # postproc pass width
    CW = 2 + 2 * NT
    O_BQ, O_BK = 0, 1
    O_A = 2
    O_B = O_A + NT
    O_J0 = win  # j0 offset inside cst16

    w2_d = nc.dram_tensor("w2", [P, 2 * DCH * MD], F8, kind="ExternalInput")
    xt_d = nc.dram_tensor("xt", [NPC, P, DCH * PROJ_CHUNK], F8, kind="ExternalInput")
    cs_d = nc.dram_tensor("cst", [P, CW], F32, kind="ExternalInput")
    c16_d = nc.dram_tensor("cst16", [P, win + TPQ * win], BF16, kind="ExternalInput")
    y_d = nc.dram_tensor("y", [P, NT], F32, kind="ExternalOutput")

    with tile.TileContext(nc) as tc:
        with (
            tc.tile_pool(name="const", bufs=1) as const,
            tc.tile_pool(name="psum_proj", bufs=2, space="PSUM") as psum_proj,
            tc.tile_pool(name="psum_band", bufs=3, space="PSUM") as psum_band,
            tc.tile_pool(name="band_sp", bufs=3) as sp_pool,
            tc.tile_pool(name="band_e", bufs=3) as e_pool,
            tc.tile_pool(name="comb", bufs=1) as comb,
        ):
            # ---- engine warmups (run while DMAs are in flight) ----
            wtile = const.tile([P, 64], BF16, tag="warm_w")
            nc.vector.memset(wtile, 0.0)
            wact_in = const.tile([P, 1], F32, tag="warm_a")
            nc.vector.memset(wact_in, 0.0)
            for _ in range(N_WARM):
                wps = psum_band.tile([P, W4], F32, tag="band")
                nc.tensor.matmul(
                    wps[:64, :64], lhsT=wtile[:, :64], rhs=wtile, start=True, stop=True
                )
            wact_out = const.tile([P, 1], F32, tag="warm_ao")
            nc.scalar.activation(out=wact_out, in_=wact_in, func=AF.Exp)

            # ---- input DMAs, ordered by need time ----
            # sync queue: xt0, xt2, cst16, cst (+ the y output later)
            # scalar queue: w2, xt1, xt3
            xts = []
            for i in range(NPC):
                t = const.tile([P, DCH * PROJ_CHUNK], F8, tag=f"xt{i}")
                xts.append(t)
            nc.sync.dma_start(out=xts[0], in_=xt_d[0])
            w2_s = const.tile([P, 2 * DCH * MD], F8, tag="w2")
            nc.scalar.dma_start(out=w2_s, in_=w2_d[:, :])
            nc.sync.dma_start(out=xts[1], in_=xt_d[1])
            nc.scalar.dma_start(out=xts[2], in_=xt_d[2])
            c16_s = const.tile([P, win + W4], BF16, tag="cst16")
            nc.sync.dma_start(out=c16_s, in_=c16_d[:, :])
            nc.scalar.dma_start(out=xts[3], in_=xt_d[3])
            cs_s = const.tile([P, CW], F32, tag="cst")
            nc.sync.dma_start(out=cs_s, in_=cs_d[:, :])

            # q is cols [0, N), k is cols [N+PAD, 2N+PAD) with zeroed
            # PAD-wide borders so unclamped windows read exp-neutral zeros
            qkT = const.tile([P, 2 * N + 2 * PAD], BF16, tag="qkT")
            nc.gpsimd.memset(qkT[:, N : N + PAD], 0.0)
            nc.gpsimd.memset(qkT[:, 2 * N + PAD :], 0.0)
            # per-quad sums, 8 cols per quad: e(4q..4q+3) | ec(4q..4q+3)
            sums8 = const.tile([P, 2 * TPQ * NQUAD], F32, tag="sums8")
            outv = const.tile([P, NT], F32, tag="outv")

            # ---- projections ----
            def emit_proj(n4):
                ps = psum_proj.tile([P, 2 * PROJ_CHUNK], F32, tag="proj")
                rhs = xts[n4][:, :].rearrange("p (c f) -> p c f", c=DCH)
                for pj in range(2):  # 0=q, 1=k
                    lhsT = w2_s[:, pj * DCH * MD : (pj + 1) * DCH * MD].rearrange(
                        "p (c m) -> p c m", c=DCH
                    )
                    nc.tensor.matmul(
                        ps[:, pj * PROJ_CHUNK : (pj + 1) * PROJ_CHUNK],
                        lhsT=lhsT,
                        rhs=rhs,
                        start=True,
                        stop=True,
                        perf_mode=mybir.MatmulPerfMode.DoubleRow,
                    )
                return ps

            def emit_evict(n4, ps, pj, eng):
                lo = n4 * PROJ_CHUNK
                src = ps[:, pj * PROJ_CHUNK : (pj + 1) * PROJ_CHUNK]
                ko = pj * (N + PAD)
                dst = qkT[:, ko + lo : ko + lo + PROJ_CHUNK]
                if has_bias:
                    nc.scalar.activation(
                        out=dst, in_=src, func=AF.Identity,
                        bias=cs_s[:, O_BQ + pj : O_BQ + pj + 1], scale=1.0,
                    )
                elif eng == "act":
                    nc.scalar.activation(out=dst, in_=src, func=AF.Identity)
                else:
                    nc.vector.tensor_copy(dst, src)

            # ---- band quad: tiles 4q..4q+3 share one [P, 4*win] pass ----
            quad_ps = {}

            def emit_quad_mm(q):
                ps = psum_band.tile([P, W4], F32, tag="band")
                for tb in range(TPQ):
                    t = TPQ * q + tb
                    for hb in range(2):  # 64-row half on partitions
                        ws = ws2[2 * t + hb]
                        nc.tensor.matmul(
                            ps[hb * HR : (hb + 1) * HR, tb * win : (tb + 1) * win],
                            lhsT=qkT[:, t * P + hb * HR : t * P + (hb + 1) * HR],
                            rhs=qkT[:, N + PAD + ws : N + PAD + ws + win],
                            start=True,
                            stop=True,
                        )
                quad_ps[q] = ps

            quad_eej = {}

            def emit_quad_red(q):
                eej = quad_eej.pop(q)
                nc.vector.tensor_reduce(
                    out=sums8[:, 2 * TPQ * q : 2 * TPQ * (q + 1)],
                    in_=eej[:].rearrange("p (f w) -> p f w", w=win),
                    axis=mybir.AxisListType.X,
                    op=AL.add,
                )

            def emit_quad_post(q):
                ps = quad_ps.pop(q)
                sp = sp_pool.tile([P, W4], F32, tag="sp")
                pat_b = (
                    c16_s[:, :win].unsqueeze(1).broadcast_to([P, TPQ, win])
                )
                nc.vector.tensor_mul(
                    sp[:].rearrange("p (f w) -> p f w", w=win),
                    ps[:].rearrange("p (f w) -> p f w", w=win),
                    pat_b,
                )
                # packed e | ej tile: exp writes [:, :W4], ej in [:, W4:]
                eej = e_pool.tile([P, 2 * W4], BF16, tag="eej")
                nc.scalar.activation(out=eej[:, :W4], in_=sp, func=AF.Exp)
                nc.vector.tensor_mul(
                    eej[:, W4:], eej[:, :W4], c16_s[:, O_J0 : O_J0 + W4]
                )
                quad_eej[q] = eej

            # ---- combine: out = (A + sum_ec + B*sum_e)/(N-win+sum_e) ----
            # sums8 views: per quad 8 cols [e x4, ec x4]
            def sview(sl, off):
                q0, q1 = sl.start // TPQ, sl.stop // TPQ
                return sums8[:, 2 * TPQ * q0 : 2 * TPQ * q1].rearrange(
                    "p (q eight) -> p q eight", eight=2 * TPQ
                )[:, :, off : off + TPQ]

            def cview(o, sl):
                return cs_s[:, o + sl.start : o + sl.stop].rearrange(
                    "p (q four) -> p q four", four=TPQ
                )

            def comb_pre(sl):
                w = sl.stop - sl.start
                se = sview(sl, 0)
                t0 = comb.tile([P, w], F32, tag=f"t0{sl.start}")
                nc.gpsimd.tensor_scalar_add(
                    t0[:].rearrange("p (q four) -> p q four", four=TPQ),
                    se,
                    float(N - win),
                )
                rec = comb.tile([P, w], F32, tag=f"rec{sl.start}")
                nc.vector.reciprocal(rec, t0)
                tmp = comb.tile([P, w], F32, tag=f"tmp{sl.start}")
                nc.gpsimd.tensor_mul(
                    tmp[:].rearrange("p (q four) -> p q four", four=TPQ),
                    cview(O_B, sl),
                    se,
                )
                return rec, tmp

            def comb_post(sl, rec, tmp):
                w = sl.stop - sl.start
                sec = sview(sl, TPQ)
                num = comb.tile([P, w], F32, tag=f"num{sl.start}")
                nc.gpsimd.tensor_add(
                    num[:].rearrange("p (q four) -> p q four", four=TPQ),
                    cview(O_A, sl),
                    sec,
                )
                num2 = comb.tile([P, w], F32, tag=f"num2{sl.start}")
                nc.gpsimd.tensor_add(num2, num, tmp)
                nc.gpsimd.tensor_mul(outv[:, sl], num2, rec)
                nc.sync.dma_start(out=y_d[:, sl], in_=outv[:, sl])

            # ---- schedule ----
            slA, slB = slice(0, 8), slice(8, NT)

            ps0 = emit_proj(0)
            emit_evict(0, ps0, 1, "act")
            emit_evict(0, ps0, 0, "dve")
            ps1 = emit_proj(1)
            emit_evict(1, ps1, 1, "act")
            emit_evict(1, ps1, 0, "dve")
            ps2 = emit_proj(2)
            emit_evict(2, ps2, 1, "act")
            emit_quad_mm(0)
            ps3 = emit_proj(3)
            emit_evict(2, ps2, 0, "dve")
            emit_evict(3, ps3, 1, "act")
            emit_evict(3, ps3, 0, "act")
            emit_quad_post(0)
            emit_quad_mm(1)
            emit_quad_red(0)
            emit_quad_post(1)
            emit_quad_mm(2)
            emit_quad_post(2)
            emit_quad_red(1)
            emit_quad_mm(3)
            emit_quad_post(3)
            emit_quad_red(2)
            recA, tmpA = comb_pre(slA)
            comb_post(slA, recA, tmpA)
            emit_quad_red(3)
            recB, tmpB = comb_pre(slB)
            comb_post(slB, recB, tmpB)

    nc.finalize()
    return nc


def kernel(x, Wq, bq, Wk, bk, prior_mean, prior_std):
    global last_run
    x = np.asarray(x, dtype=np.float32)
    Wq = np.asarray(Wq, dtype=np.float32)
    Wk = np.asarray(Wk, dtype=np.float32)
    bq = np.asarray(bq, dtype=np.float32)
    bk = np.asarray(bk, dtype=np.float32)
    has_bias = bool(np.any(bq) or np.any(bk))

    prior, dlo, dhi = _plan_band(
        float(np.asarray(prior_mean)[0]), float(np.asarray(prior_std)[0])
    )
    win, ws2, key_vals, key_idx = _window_geometry(dlo, dhi)
    n_pat = len(key_vals)

    key = (win, tuple(ws2), tuple(key_idx), has_bias)
    if key not in _cache:
        _cache[key] = _build(win, ws2, key_idx, n_pat, has_bias)
    nc = _cache[key]

    bf = ml_dtypes.bfloat16
    f8 = ml_dtypes.float8_e4m3fn
    scale = np.float32(MD**-0.5)
    W4 = TPQ * win

    # prior*scale pattern: with unclamped windows every tile block is the
    # same [P, win] pattern: dm = c + dlo - extra//2 + HR*(p>=HR) - p
    p_idx = np.arange(P)[:, None]
    c_idx = np.arange(win)[None, :]
    rel0 = ws2[0]  # = dlo - extra//2
    relcol = np.where(np.arange(P) < HR, rel0, rel0 + HR)[:, None]
    dm = c_idx + relcol - p_idx
    pmat = np.where(
        (dm >= dlo) & (dm <= dhi), prior[dm + N - 1] * scale, np.float32(0.0)
    ).astype(np.float32)

    sumj_all = float(N * (N - 1) // 2)
    half_sel = np.arange(P) >= HR
    ii = (np.arange(P)[:, None] + P * np.arange(NT)[None, :]).astype(np.float32)
    wsv = np.zeros((P, NT), np.float32)
    for t in range(NT):
        wsv[:, t] = np.where(half_sel, float(ws2[2 * t + 1]), float(ws2[2 * t]))
    c1 = sumj_all - (win * wsv + win * (win - 1) // 2)
    A = c1 - ii * float(N - win)
    Bv = wsv - ii

    cst = np.ascontiguousarray(
        np.concatenate([bq.reshape(P, 1), bk.reshape(P, 1), A, Bv], axis=1).astype(
            np.float32
        )
    )
    j0quad = np.broadcast_to(
        np.tile(np.arange(win, dtype=np.float32), TPQ)[None, :], (P, TPQ * win)
    )
    cst16 = np.ascontiguousarray(np.concatenate([pmat, j0quad], axis=1).astype(bf))

    # weights: wq chunks then wk chunks, [P, 4*MD] fp8
    wq_h = Wq.reshape(DCH, P, MD).transpose(1, 0, 2).reshape(P, DCH * MD)
    wk_h = Wk.reshape(DCH, P, MD).transpose(1, 0, 2).reshape(P, DCH * MD)
    w2_h = np.clip(np.concatenate([wq_h, wk_h], axis=1), -240, 240)
    w2_h = np.ascontiguousarray(w2_h).astype(f8)

    in_maps = []
    for core in range(NCORES):
        xb = x[core]  # [N, D]
        # xt[n4, p, c*512 + j] = x[n4*512 + j, c*128 + p]
        xt_h = np.ascontiguousarray(
            np.clip(xb.T, -240, 240)
            .reshape(DCH, P, NPC, PROJ_CHUNK)
            .transpose(2, 1, 0, 3)
            .reshape(NPC, P, DCH * PROJ_CHUNK)
        ).astype(f8)
        in_maps.append({"xt": xt_h, "w2": w2_h, "cst": cst, "cst16": cst16})

    res = run_bass_kernel_spmd(nc, in_maps, list(range(NCORES)))
    last_run = (nc, in_maps)
    # y[p, t] = out[128t + p]  ->  out = y.T.flatten()
    out = np.stack(
        [res.results[c]["y"].T.reshape(-1) for c in range(NCORES)], axis=0
    )
    return out.astype(np.float32)
